# revision 4
# baseline (speedup 1.0000x reference)
"""Trainium2 Bass kernel for the 2-expert ResNet20 MoE (nn_MoE_48095043780864).

Strategy (pure data parallel, 8 cores x 64 images):
  - Channel counts (3/16/32/64) are far below the 128-wide PE array, so we
    pack G images' channels into the partition dim (G=8/4/2 per stage) and
    use block-diagonal stationary matrices: every conv = 9 shifted-tap
    matmuls [128,128]x[128,<=512] accumulated in PSUM fp32.
  - Activations live in SBUF as fp16, padded spatially (pad=1) with flat
    per-image layout + 64-column guards so taps are plain offset reads.
  - Stride-2 convs (jax SAME pads (0,1) on even input) are two matmul sets
    (A: j'=j from input groups g, B: j'=j+J from groups g+G') with strided
    access patterns; they also perform the G -> G/2 repacking for free.
  - BN is folded into weights (scale) + activation bias; ReLU+bias fused in
    the ScalarE epilogue; residual adds on VectorE; pad rings re-zeroed by
    GpSimd memsets.
  - Gate conv runs fp16 like the experts; gate softmax/soft-topk, the
    gate-weighted combine and the load-balance loss run on host in fp32
    (tiny), which keeps the sharp sigmoids bit-comparable to the reference.
"""
import numpy as np

# ---------------- static plan ----------------
S1 = dict(C=16, G=8, J=8, H=32, Hp=34)
S2 = dict(C=32, G=4, J=16, H=16, Hp=18)
S3 = dict(C=64, G=2, J=32, H=8, Hp=10)
STAGES = {1: S1, 2: S2, 3: S3}
GUARD = 64
N_CORES = 8
N_PER_CORE = 64


def _flat(s):
    return s['J'] * s['Hp'] * s['Hp']


def _plan():
    """Static description of every conv unit + table offsets."""
    units = []
    wcol = [0]
    bcol = [0]

    def conv(stage, src, dst, kin, add=None, tag=''):
        u = dict(kind='conv', stage=stage, src=src, dst=dst, kin=kin, add=add,
                 ntaps=9, wcol=wcol[0], bcol=bcol[0], tag=tag)
        wcol[0] += 9 * 128
        bcol[0] += 1
        units.append(u)

    def trans(si, so, src, dst, ntaps, relu, tag=''):
        u = dict(kind='trans', si=si, so=so, src=src, dst=dst, ntaps=ntaps,
                 relu=relu, wcol=wcol[0], bcol=bcol[0], tag=tag)
        wcol[0] += 2 * ntaps * 128
        bcol[0] += 1
        units.append(u)

    conv(1, 'X', 'B', 24, tag='gate')          # gate conv (epilogue relu)
    for e in (1, 2):
        p = f'e{e}.'
        conv(1, 'X', 'A', 24, tag=p + 'conv0')
        h = 'A'
        for blk in range(3):                    # stage1 blocks
            conv(1, h, 'B', 128, tag=f'{p}b{blk}c1')
            nxt = 'C' if h == 'A' else 'A'
            conv(1, 'B', nxt, 128, add=h, tag=f'{p}b{blk}c2')
            h = nxt
        trans(1, 2, h, 'D', 9, True, tag=p + 'b3c1')     # 16->32 s2
        trans(1, 2, h, 'F', 1, False, tag=p + 'b3proj')
        conv(2, 'D', 'E', 128, add='F', tag=p + 'b3c2')
        h = 'E'
        for blk in (4, 5):                      # stage2 blocks
            conv(2, h, 'D', 128, tag=f'{p}b{blk}c1')
            nxt = 'F' if h == 'E' else 'E'
            conv(2, 'D', nxt, 128, add=h, tag=f'{p}b{blk}c2')
            h = nxt
        assert h == 'E'
        trans(2, 3, h, 'G', 9, True, tag=p + 'b6c1')     # 32->64 s2
        trans(2, 3, h, 'I', 1, False, tag=p + 'b6proj')
        conv(3, 'G', 'H', 128, add='I', tag=p + 'b6c2')
        h = 'H'
        for blk in (7, 8):                      # stage3 blocks
            conv(3, h, 'G', 128, tag=f'{p}b{blk}c1')
            nxt = 'I' if h == 'H' else 'H'
            conv(3, 'G', nxt, 128, add=h, tag=f'{p}b{blk}c2')
            h = nxt
        assert h == 'H'
    nb = bcol[0] + 3            # + fcb1, fcb2, lb
    return units, wcol[0], nb


UNITS, WCOLS, NBIAS = _plan()
BCOL_FCB1 = NBIAS - 3
BCOL_FCB2 = NBIAS - 2
BCOL_LB = NBIAS - 1
BUF_STAGE = dict(A=1, B=1, C=1, D=2, E=2, F=2, G=3, H=3, I=3)


def _s1_ids():
    return np.array([[8 * g + j for j in range(8)] for g in range(8)])


def _s3_ids():
    i1 = _s1_ids()
    i2 = np.zeros((4, 16), int)
    for g in range(4):
        i2[g, :8] = i1[g]
        i2[g, 8:] = i1[g + 4]
    i3 = np.zeros((2, 32), int)
    for g in range(2):
        i3[g, :16] = i2[g]
        i3[g, 16:] = i2[g + 2]
    return i3


# ---------------- host-side weight table builders ----------------
def _np32(a):
    return np.asarray(a, np.float32)


def _fold(p, wk, gk, bk):
    w = _np32(p[wk])
    g = _np32(p[gk])
    return w * g[:, None, None, None], _np32(p[bk])


def _blockdiag(Wf, t, gmap, I_, O):
    out = np.zeros((128, 128), np.float32)
    blk = Wf[:, :, t // 3, t % 3].T      # [I, O]
    for gi, go in gmap:
        out[gi * I_:(gi + 1) * I_, go * O:(go + 1) * O] = blk
    return out


def _build_tables(gp, P1, P2):
    wts = np.zeros((128, WCOLS), np.float32)
    bias = np.zeros((128, NBIAS), np.float32)
    fcw = np.zeros((128, 56), np.float32)

    def expert_weight_seq(P):
        """yields (Wfold, bias_vec, gmap_or_transmaps) per unit in plan order"""
        W0, b0 = _fold(P, 'w0', 'g0', 'b0')
        yield ('conv', W0, b0, 8)
        blocks = P['blocks']
        for blk in range(3):
            p, _ = blocks[blk]
            yield ('conv', *_fold(p, 'w1', 'g1', 'b1'), 8)
            yield ('conv', *_fold(p, 'w2', 'g2', 'b2'), 8)
        p, _ = blocks[3]
        W1, b1 = _fold(p, 'w1', 'g1', 'b1')
        Wp, bp = _fold(p, 'wp', 'gp', 'bp')
        yield ('trans', W1, b1, (4, [(g, g) for g in range(4)],
                                 [(g + 4, g) for g in range(4)]))
        yield ('trans', Wp, bp, (4, [(g, g) for g in range(4)],
                                 [(g + 4, g) for g in range(4)]))
        yield ('conv', *_fold(p, 'w2', 'g2', 'b2'), 4)
        for blk in (4, 5):
            p, _ = blocks[blk]
            yield ('conv', *_fold(p, 'w1', 'g1', 'b1'), 4)
            yield ('conv', *_fold(p, 'w2', 'g2', 'b2'), 4)
        p, _ = blocks[6]
        W1, b1 = _fold(p, 'w1', 'g1', 'b1')
        Wp, bp = _fold(p, 'wp', 'gp', 'bp')
        yield ('trans', W1, b1, (2, [(g, g) for g in range(2)],
                                 [(g + 2, g) for g in range(2)]))
        yield ('trans', Wp, bp, (2, [(g, g) for g in range(2)],
                                 [(g + 2, g) for g in range(2)]))
        yield ('conv', *_fold(p, 'w2', 'g2', 'b2'), 2)
        for blk in (7, 8):
            p, _ = blocks[blk]
            yield ('conv', *_fold(p, 'w1', 'g1', 'b1'), 2)
            yield ('conv', *_fold(p, 'w2', 'g2', 'b2'), 2)

    def gate_weight_seq():
        cw = _np32(gp['cw'])
        cb = _np32(gp['cb'])
        yield ('conv', cw, cb, 8)

    seq = list(gate_weight_seq()) + list(expert_weight_seq(P1)) \
        + list(expert_weight_seq(P2))
    assert len(seq) == len(UNITS)
    for u, (kind, Wf, bv, extra) in zip(UNITS, seq):
        assert kind == u['kind'], (u, kind)
        O, I_ = Wf.shape[0], Wf.shape[1]
        if kind == 'conv':
            G = extra
            gmap = [(g, g) for g in range(G)]
            for t in range(u['ntaps']):
                wts[:, u['wcol'] + t * 128: u['wcol'] + (t + 1) * 128] = \
                    _blockdiag(Wf, t, gmap, I_, O)
            bias[:, u['bcol']] = np.tile(bv, G)
        else:
            Gout, gA, gB = extra
            nt = u['ntaps']
            for t in range(nt):
                tt = t if nt == 9 else 0
                wts[:, u['wcol'] + t * 128: u['wcol'] + (t + 1) * 128] = \
                    _blockdiag(Wf, tt, gA, I_, O)
                c0 = u['wcol'] + (nt + t) * 128
                wts[:, c0:c0 + 128] = _blockdiag(Wf, tt, gB, I_, O)
            bias[:, u['bcol']] = np.tile(bv, Gout)

    # fc weights (fold mean 1/64) + gate head (fold mean 1/1024)
    for e, P in ((0, P1), (1, P2)):
        fw = _np32(P['fcw']) / 64.0      # [10, 64]
        for g in range(2):
            fcw[g * 64:(g + 1) * 64, e * 20 + g * 10: e * 20 + (g + 1) * 10] = fw.T
        bias[:20, BCOL_FCB1 + e] = np.tile(_np32(P['fcb']), 2)
    lw = _np32(gp['lw']) / 1024.0        # [2, 16]
    for g in range(8):
        fcw[g * 16:(g + 1) * 16, 40 + g * 2:40 + (g + 1) * 2] = lw.T
    bias[:16, BCOL_LB] = np.tile(_np32(gp['lb']), 8)
    return wts.astype(np.float16), bias, fcw


def _pack_x(x_core):
    """[64,3,32,32] fp32 -> [24, F1] fp16 stage1-packed padded"""
    out = np.zeros((24, S1['J'], 34, 34), np.float16)
    ids = _s1_ids()
    for g in range(8):
        out[g * 3:(g + 1) * 3, :, 1:33, 1:33] = \
            x_core[ids[g]].transpose(1, 0, 2, 3)
    return out.reshape(24, _flat(S1))


# ---------------- bass program ----------------
_NC_CACHE = {}


def _build_nc():
    if 'nc' in _NC_CACHE:
        return _NC_CACHE['nc']
    import concourse.bacc as bacc
    import concourse.mybir as mybir
    import concourse.tile as tile
    from contextlib import ExitStack

    f16, f32 = mybir.dt.float16, mybir.dt.float32
    RELU = mybir.ActivationFunctionType.Relu
    IDENT = mybir.ActivationFunctionType.Identity
    ADD = mybir.AluOpType.add
    XY = mybir.AxisListType.XY

    nc = bacc.Bacc("TRN2", target_bir_lowering=False, debug=False,
                   enable_asserts=False, num_devices=N_CORES)
    xin = nc.dram_tensor('xin', [24, _flat(S1)], f16, kind='ExternalInput')
    wdr = nc.dram_tensor('wts', [128, WCOLS], f16, kind='ExternalInput')
    bdr = nc.dram_tensor('bias', [128, NBIAS], f32, kind='ExternalInput')
    fdr = nc.dram_tensor('fcw', [128, 56], f32, kind='ExternalInput')
    y1d = nc.dram_tensor('y1', [20, 32], f32, kind='ExternalOutput')
    y2d = nc.dram_tensor('y2', [20, 32], f32, kind='ExternalOutput')
    gld = nc.dram_tensor('glog', [16, 8], f32, kind='ExternalOutput')

    with tile.TileContext(nc) as tc, ExitStack() as ctx:
        persist = ctx.enter_context(tc.tile_pool(name='persist', bufs=1))
        wpool = ctx.enter_context(tc.tile_pool(name='wpool', bufs=3))
        pspool = ctx.enter_context(
            tc.tile_pool(name='psum', bufs=8, space='PSUM'))
        tpool = ctx.enter_context(tc.tile_pool(name='tmp', bufs=4))

        X = persist.tile([24, _flat(S1) + 2 * GUARD], f16, tag='X')
        bufs = {}
        for nm, st in BUF_STAGE.items():
            bufs[nm] = persist.tile(
                [128, _flat(STAGES[st]) + 2 * GUARD], f16, tag=nm, name=nm)
        bt = persist.tile([128, NBIAS], f32, tag='bias')
        ft = persist.tile([128, 56], f32, tag='fcw')

        nc.sync.dma_start(bt[:], bdr[:])
        nc.sync.dma_start(ft[:], fdr[:])
        nc.sync.dma_start(X[:, GUARD:GUARD + _flat(S1)], xin[:])
        nc.gpsimd.memset(X[:, 0:GUARD], 0.0)
        nc.gpsimd.memset(X[:, GUARD + _flat(S1):], 0.0)
        for nm, st in BUF_STAGE.items():
            F = _flat(STAGES[st])
            nc.gpsimd.memset(bufs[nm][:, 0:GUARD], 0.0)
            nc.gpsimd.memset(bufs[nm][:, GUARD + F:], 0.0)

        def pad_memsets(dst, s):
            F = _flat(s)
            Hp = s['Hp']
            v = dst[:, GUARD:GUARD + F].rearrange(
                'p (j y x) -> p j y x', j=s['J'], y=Hp, x=Hp)
            nc.gpsimd.memset(v[:, :, 0:Hp:Hp - 1, :], 0.0)
            nc.gpsimd.memset(v[:, :, :, 0:Hp:Hp - 1], 0.0)

        def emit_conv(u):
            s = STAGES[u['stage']]
            F, Hp = _flat(s), s['Hp']
            kin = u['kin']
            wt = wpool.tile([128, 9 * 128], f16, tag='w', name='wconv')
            nc.sync.dma_start(wt[:], wdr[:, u['wcol']:u['wcol'] + 9 * 128])
            IN = bufs[u['src']] if u['src'] != 'X' else X
            OUT = bufs[u['dst']]
            b_ap = bt[:, u['bcol']:u['bcol'] + 1]
            ntiles = (F + 511) // 512
            for i in range(ntiles):
                n = min(512, F - i * 512)
                ps = pspool.tile([128, 512], f32, tag='ps', name='ps')
                for t in range(9):
                    dy, dx = t // 3 - 1, t % 3 - 1
                    off = GUARD + dy * Hp + dx + i * 512
                    nc.tensor.matmul(
                        ps[:, :n],
                        wt[0:kin, t * 128:t * 128 + 128],
                        IN[0:kin, off:off + n],
                        start=(t == 0), stop=(t == 8))
                dst = OUT[:, GUARD + i * 512:GUARD + i * 512 + n]
                if u['add'] is None:
                    nc.scalar.activation(dst, ps[:, :n], RELU, bias=b_ap)
                else:
                    AD = bufs[u['add']]
                    tmp = tpool.tile([128, 512], f32, tag='tmp', name='tmp')
                    nc.vector.tensor_add(
                        tmp[:, :n], ps[:, :n],
                        AD[:, GUARD + i * 512:GUARD + i * 512 + n])
                    nc.scalar.activation(dst, tmp[:, :n], RELU, bias=b_ap)
            pad_memsets(OUT, s)

        def emit_trans(u):
            si, so = STAGES[u['si']], STAGES[u['so']]
            Ji, Ho, Hpo = si['J'], so['H'], so['Hp']
            nt = u['ntaps']
            wt = wpool.tile([128, 2 * nt * 128], f16, tag='w', name='wtrans')
            nc.sync.dma_start(wt[:], wdr[:, u['wcol']:u['wcol'] + 2 * nt * 128])
            IN, OUT = bufs[u['src']], bufs[u['dst']]
            b_ap = bt[:, u['bcol']:u['bcol'] + 1]
            vin = IN[:, GUARD:GUARD + _flat(si)].rearrange(
                'p (j y x) -> p j y x', j=Ji, y=si['Hp'], x=si['Hp'])
            vout = OUT[:, GUARD:GUARD + _flat(so)].rearrange(
                'p (j y x) -> p j y x', j=so['J'], y=Hpo, x=Hpo)
            m = 512 // (Ho * Ho)          # images per psum tile
            for st_i in range(2):          # set A then B
                for k in range(Ji // m):
                    j0 = k * m
                    n = m * Ho * Ho
                    ps = pspool.tile([128, 512], f32, tag='ps', name='ps')
                    for t in range(nt):
                        ky, kx = (t // 3, t % 3) if nt == 9 else (0, 0)
                        rhs = vin[:, j0:j0 + m,
                                  1 + ky:1 + ky + 2 * Ho - 1:2,
                                  1 + kx:1 + kx + 2 * Ho - 1:2]
                        c0 = u['wcol'] + (st_i * nt + t) * 128
                        nc.tensor.matmul(
                            ps[:, :n], wt[:, c0 - u['wcol']:c0 - u['wcol'] + 128],
                            rhs, start=(t == 0), stop=(t == nt - 1))
                    jo = st_i * Ji + j0
                    dst = vout[:, jo:jo + m, 1:1 + Ho, 1:1 + Ho]
                    nc.scalar.activation(
                        dst, ps[:, :n], RELU if u['relu'] else IDENT, bias=b_ap)
            pad_memsets(OUT, so)

        def emit_pool_fc(e, src, ydst):
            v = bufs[src][:, GUARD:GUARD + _flat(S3)].rearrange(
                'p (j y x) -> p j y x', j=32, y=10, x=10)
            pooled = tpool.tile([128, 32], f32, tag=f'pool{e}', name=f'pool{e}')
            nc.vector.tensor_reduce(
                pooled[:], v[:, :, 1:9, 1:9], axis=XY, op=ADD)
            ps = pspool.tile([128, 512], f32, tag='ps', name='ps')
            nc.tensor.matmul(ps[0:20, 0:32], ft[:, e * 20:e * 20 + 20],
                             pooled[:], start=True, stop=True)
            ysb = tpool.tile([20, 32], f32, tag=f'y{e}', name=f'ysb{e}')
            nc.scalar.activation(
                ysb[:], ps[0:20, 0:32], IDENT,
                bias=bt[0:20, BCOL_FCB1 + e:BCOL_FCB1 + e + 1])
            nc.sync.dma_start(ydst[:], ysb[:])

        def emit_gate_head():
            v = bufs['B'][:, GUARD:GUARD + _flat(S1)].rearrange(
                'p (j y x) -> p j y x', j=8, y=34, x=34)
            pooled = tpool.tile([128, 8], f32, tag='poolg', name='poolg')
            nc.vector.tensor_reduce(
                pooled[:], v[:, :, 1:33, 1:33], axis=XY, op=ADD)
            ps = pspool.tile([128, 512], f32, tag='ps', name='ps')
            nc.tensor.matmul(ps[0:16, 0:8], ft[:, 40:56], pooled[:],
                             start=True, stop=True)
            gsb = tpool.tile([16, 8], f32, tag='yg', name='gsb')
            nc.scalar.activation(gsb[:], ps[0:16, 0:8], IDENT,
                                 bias=bt[0:16, BCOL_LB:BCOL_LB + 1])
            nc.sync.dma_start(gld[:], gsb[:])

        ui = 0
        # gate conv
        emit_conv(UNITS[ui]); ui += 1
        emit_gate_head()
        for e, ydst in ((0, y1d), (1, y2d)):
            while ui < len(UNITS) and UNITS[ui]['tag'].startswith(f'e{e+1}.'):
                u = UNITS[ui]
                if u['kind'] == 'conv':
                    emit_conv(u)
                else:
                    emit_trans(u)
                ui += 1
            emit_pool_fc(e, 'H', ydst)
        assert ui == len(UNITS)

    nc.compile()
    _NC_CACHE['nc'] = nc
    return nc


# ---------------- host orchestration ----------------
def _soft_topk(logits):
    logits = logits.astype(np.float32)
    m = logits.max(-1, keepdims=True)
    e = np.exp(logits - m)
    s = e / e.sum(-1, keepdims=True)
    diff = s[:, :, None] - s[:, None, :]
    with np.errstate(over='ignore'):
        sigma = 1.0 / (1.0 + np.exp(diff / np.float32(1e-5)))
    r = 1.0 + (sigma.sum(-1) - np.float32(0.5))
    a = 1.0 / (1.0 + np.exp(-(np.float32(2.5) - r) / np.float32(1e-5)))
    return a * s


def kernel(x, train, gate_params, expert_params1, expert_params2):
    del train
    from concourse.bass_utils import run_bass_kernel_spmd

    x = np.asarray(x, np.float32)

    def tonp(tree):
        if isinstance(tree, dict):
            return {k: tonp(v) for k, v in tree.items()}
        if isinstance(tree, (list, tuple)):
            return type(tree)(tonp(v) for v in tree)
        return np.asarray(tree, np.float32)

    gp = tonp(gate_params)
    P1 = tonp(expert_params1)
    P2 = tonp(expert_params2)

    wts, bias, fcw = _build_tables(gp, P1, P2)
    nc = _build_nc()

    in_maps = []
    for ci in range(N_CORES):
        xc = x[ci * N_PER_CORE:(ci + 1) * N_PER_CORE]
        in_maps.append(dict(xin=_pack_x(xc), wts=wts, bias=bias, fcw=fcw))

    res = run_bass_kernel_spmd(nc, in_maps, core_ids=list(range(N_CORES)))
    outs = res.results

    i1, i3 = _s1_ids(), _s3_ids()
    NB = x.shape[0]
    y = np.zeros((NB, 10), np.float32)
    logits = np.zeros((NB, 2), np.float32)
    for ci in range(N_CORES):
        o = outs[ci]
        ys1 = np.zeros((N_PER_CORE, 10), np.float32)
        ys2 = np.zeros((N_PER_CORE, 10), np.float32)
        gl = np.zeros((N_PER_CORE, 2), np.float32)
        for g in range(2):
            for j in range(32):
                ys1[i3[g, j]] = o['y1'][g * 10:(g + 1) * 10, j]
                ys2[i3[g, j]] = o['y2'][g * 10:(g + 1) * 10, j]
        for g in range(8):
            for j in range(8):
                gl[i1[g, j]] = o['glog'][g * 2:(g + 1) * 2, j]
        gates = _soft_topk(gl)
        y[ci * N_PER_CORE:(ci + 1) * N_PER_CORE] = \
            gates[:, 0:1] * ys1 + gates[:, 1:2] * ys2
        logits[ci * N_PER_CORE:(ci + 1) * N_PER_CORE] = gl

    gates_all = _soft_topk(logits)
    importance = gates_all.sum(0)
    mean = importance.mean()
    var = importance.var(ddof=1)
    loss = np.float32(0.01) * var / (mean * mean + np.float32(1e-10))
    return y, np.float32(loss)


# revision 9
# speedup vs baseline: 1.3328x; 1.3328x over previous
"""Trainium2 Bass kernel for the 2-expert ResNet20 MoE (nn_MoE_48095043780864).

Strategy (pure data parallel, 8 cores x 64 images):
  - Channel counts (3/16/32/64) are far below the 128-wide PE array, so we
    pack G images' channels into the partition dim (G=8/4/2 per stage) and
    use block-diagonal stationary matrices: every conv = 9 shifted-tap
    matmuls [128,128]x[128,<=512] accumulated in PSUM fp32.
  - Activations live in SBUF as fp16, padded spatially (pad=1) with flat
    per-image layout + 64-column guards so taps are plain offset reads.
  - Stride-2 convs (jax SAME pads (0,1) on even input) are two matmul sets
    (A: j'=j from input groups g, B: j'=j+J from groups g+G') with strided
    access patterns; they also perform the G -> G/2 repacking for free.
  - BN is folded into weights (scale) + activation bias; ReLU+bias fused in
    the ScalarE epilogue; residual adds on VectorE; pad rings re-zeroed by
    GpSimd memsets.
  - Gate conv runs fp16 like the experts; gate softmax/soft-topk, the
    gate-weighted combine and the load-balance loss run on host in fp32
    (tiny), which keeps the sharp sigmoids bit-comparable to the reference.
"""
import numpy as np

# ---------------- static plan ----------------
S1 = dict(C=16, G=8, J=8, H=32, Hp=34)
S2 = dict(C=32, G=4, J=16, H=16, Hp=18)
S3 = dict(C=64, G=2, J=32, H=8, Hp=10)
STAGES = {1: S1, 2: S2, 3: S3}
GUARD = 64
N_CORES = 8
N_PER_CORE = 64


def _flat(s):
    return s['J'] * s['Hp'] * s['Hp']


def _plan():
    """Static description of every conv unit + table offsets."""
    units = []
    wcol = [0]
    bcol = [0]

    def conv(stage, src, dst, kin, add=None, tag=''):
        packed = (kin == 24)
        u = dict(kind='conv', stage=stage, src=src, dst=dst, kin=kin, add=add,
                 ntaps=(3 if packed else 9), packed=packed,
                 wcol=wcol[0], bcol=bcol[0], tag=tag)
        wcol[0] += u['ntaps'] * 128
        bcol[0] += 1
        units.append(u)

    def trans(si, so, src, dst, ntaps, relu, tag=''):
        u = dict(kind='trans', si=si, so=so, src=src, dst=dst, ntaps=ntaps,
                 relu=relu, wcol=wcol[0], bcol=bcol[0], tag=tag)
        wcol[0] += 2 * ntaps * 128
        bcol[0] += 1
        units.append(u)

    conv(1, 'X', 'B', 24, tag='gate')          # gate conv (epilogue relu)
    for e in (1, 2):
        p = f'e{e}.'
        conv(1, 'X', 'A', 24, tag=p + 'conv0')
        h = 'A'
        for blk in range(3):                    # stage1 blocks
            conv(1, h, 'B', 128, tag=f'{p}b{blk}c1')
            nxt = 'C' if h == 'A' else 'A'
            conv(1, 'B', nxt, 128, add=h, tag=f'{p}b{blk}c2')
            h = nxt
        trans(1, 2, h, 'D', 9, True, tag=p + 'b3c1')     # 16->32 s2
        trans(1, 2, h, 'F', 1, False, tag=p + 'b3proj')
        conv(2, 'D', 'E', 128, add='F', tag=p + 'b3c2')
        h = 'E'
        for blk in (4, 5):                      # stage2 blocks
            conv(2, h, 'D', 128, tag=f'{p}b{blk}c1')
            nxt = 'F' if h == 'E' else 'E'
            conv(2, 'D', nxt, 128, add=h, tag=f'{p}b{blk}c2')
            h = nxt
        assert h == 'E'
        trans(2, 3, h, 'G', 9, True, tag=p + 'b6c1')     # 32->64 s2
        trans(2, 3, h, 'I', 1, False, tag=p + 'b6proj')
        conv(3, 'G', 'H', 128, add='I', tag=p + 'b6c2')
        h = 'H'
        for blk in (7, 8):                      # stage3 blocks
            conv(3, h, 'G', 128, tag=f'{p}b{blk}c1')
            nxt = 'I' if h == 'H' else 'H'
            conv(3, 'G', nxt, 128, add=h, tag=f'{p}b{blk}c2')
            h = nxt
        assert h == 'H'
    nb = bcol[0] + 3            # + fcb1, fcb2, lb
    return units, wcol[0], nb


UNITS, WCOLS, NBIAS = _plan()
BCOL_FCB1 = NBIAS - 3
BCOL_FCB2 = NBIAS - 2
BCOL_LB = NBIAS - 1
BUF_STAGE = dict(A=1, B=1, C=1, D=2, E=2, F=2, G=3, H=3, I=3)


def _s1_ids():
    return np.array([[8 * g + j for j in range(8)] for g in range(8)])


def _s3_ids():
    i1 = _s1_ids()
    i2 = np.zeros((4, 16), int)
    for g in range(4):
        i2[g, :8] = i1[g]
        i2[g, 8:] = i1[g + 4]
    i3 = np.zeros((2, 32), int)
    for g in range(2):
        i3[g, :16] = i2[g]
        i3[g, 16:] = i2[g + 2]
    return i3


# ---------------- host-side weight table builders ----------------
def _np32(a):
    return np.asarray(a, np.float32)


def _fold(p, wk, gk, bk):
    w = _np32(p[wk])
    g = _np32(p[gk])
    return w * g[:, None, None, None], _np32(p[bk])


def _blockdiag(Wf, t, gmap, I_, O):
    out = np.zeros((128, 128), np.float32)
    blk = Wf[:, :, t // 3, t % 3].T      # [I, O]
    for gi, go in gmap:
        out[gi * I_:(gi + 1) * I_, go * O:(go + 1) * O] = blk
    return out


def _build_tables(gp, P1, P2):
    wts = np.zeros((128, WCOLS), np.float32)
    bias = np.zeros((128, NBIAS), np.float32)
    fcw = np.zeros((128, 56), np.float32)

    def expert_weight_seq(P):
        """yields (Wfold, bias_vec, gmap_or_transmaps) per unit in plan order"""
        W0, b0 = _fold(P, 'w0', 'g0', 'b0')
        yield ('conv', W0, b0, 8)
        blocks = P['blocks']
        for blk in range(3):
            p, _ = blocks[blk]
            yield ('conv', *_fold(p, 'w1', 'g1', 'b1'), 8)
            yield ('conv', *_fold(p, 'w2', 'g2', 'b2'), 8)
        p, _ = blocks[3]
        W1, b1 = _fold(p, 'w1', 'g1', 'b1')
        Wp, bp = _fold(p, 'wp', 'gp', 'bp')
        yield ('trans', W1, b1, (4, [(g, g) for g in range(4)],
                                 [(g + 4, g) for g in range(4)]))
        yield ('trans', Wp, bp, (4, [(g, g) for g in range(4)],
                                 [(g + 4, g) for g in range(4)]))
        yield ('conv', *_fold(p, 'w2', 'g2', 'b2'), 4)
        for blk in (4, 5):
            p, _ = blocks[blk]
            yield ('conv', *_fold(p, 'w1', 'g1', 'b1'), 4)
            yield ('conv', *_fold(p, 'w2', 'g2', 'b2'), 4)
        p, _ = blocks[6]
        W1, b1 = _fold(p, 'w1', 'g1', 'b1')
        Wp, bp = _fold(p, 'wp', 'gp', 'bp')
        yield ('trans', W1, b1, (2, [(g, g) for g in range(2)],
                                 [(g + 2, g) for g in range(2)]))
        yield ('trans', Wp, bp, (2, [(g, g) for g in range(2)],
                                 [(g + 2, g) for g in range(2)]))
        yield ('conv', *_fold(p, 'w2', 'g2', 'b2'), 2)
        for blk in (7, 8):
            p, _ = blocks[blk]
            yield ('conv', *_fold(p, 'w1', 'g1', 'b1'), 2)
            yield ('conv', *_fold(p, 'w2', 'g2', 'b2'), 2)

    def gate_weight_seq():
        cw = _np32(gp['cw'])
        cb = _np32(gp['cb'])
        yield ('conv', cw, cb, 8)

    seq = list(gate_weight_seq()) + list(expert_weight_seq(P1)) \
        + list(expert_weight_seq(P2))
    assert len(seq) == len(UNITS)
    for u, (kind, Wf, bv, extra) in zip(UNITS, seq):
        assert kind == u['kind'], (u, kind)
        O, I_ = Wf.shape[0], Wf.shape[1]
        if kind == 'conv':
            G = extra
            gmap = [(g, g) for g in range(G)]
            if u.get('packed'):
                # 3-pass tap packing: pass p<2 stacks taps 4p..4p+3 in
                # partitions (t_local*24 + g*3 + c); pass 2 is tap 8 alone.
                for p_i in range(2):
                    m = np.zeros((128, 128), np.float32)
                    for tl in range(4):
                        t = p_i * 4 + tl
                        blk = Wf[:, :, t // 3, t % 3].T   # [3, 16]
                        for g in range(8):
                            m[tl * 24 + g * 3:tl * 24 + (g + 1) * 3,
                              g * 16:(g + 1) * 16] = blk
                    wts[:, u['wcol'] + p_i * 128:u['wcol'] + (p_i + 1) * 128] = m
                wts[:, u['wcol'] + 2 * 128:u['wcol'] + 3 * 128] = \
                    _blockdiag(Wf, 8, gmap, I_, O)
            else:
                for t in range(u['ntaps']):
                    wts[:, u['wcol'] + t * 128: u['wcol'] + (t + 1) * 128] = \
                        _blockdiag(Wf, t, gmap, I_, O)
            bias[:, u['bcol']] = np.tile(bv, G)
        else:
            Gout, gA, gB = extra
            nt = u['ntaps']
            for t in range(nt):
                tt = t if nt == 9 else 0
                wts[:, u['wcol'] + t * 128: u['wcol'] + (t + 1) * 128] = \
                    _blockdiag(Wf, tt, gA, I_, O)
                c0 = u['wcol'] + (nt + t) * 128
                wts[:, c0:c0 + 128] = _blockdiag(Wf, tt, gB, I_, O)
            bias[:, u['bcol']] = np.tile(bv, Gout)

    # fc weights (fold mean 1/64) + gate head (fold mean 1/1024)
    for e, P in ((0, P1), (1, P2)):
        fw = _np32(P['fcw']) / 64.0      # [10, 64]
        for g in range(2):
            fcw[g * 64:(g + 1) * 64, e * 20 + g * 10: e * 20 + (g + 1) * 10] = fw.T
        bias[:20, BCOL_FCB1 + e] = np.tile(_np32(P['fcb']), 2)
    lw = _np32(gp['lw']) / 1024.0        # [2, 16]
    for g in range(8):
        fcw[g * 16:(g + 1) * 16, 40 + g * 2:40 + (g + 1) * 2] = lw.T
    bias[:16, BCOL_LB] = np.tile(_np32(gp['lb']), 8)
    return wts.astype(np.float16), bias, fcw


def _pack_x(x_core):
    """[64,3,32,32] fp32 -> [24, F1] fp16 stage1-packed padded"""
    out = np.zeros((24, S1['J'], 34, 34), np.float16)
    ids = _s1_ids()
    for g in range(8):
        out[g * 3:(g + 1) * 3, :, 1:33, 1:33] = \
            x_core[ids[g]].transpose(1, 0, 2, 3)
    return out.reshape(24, _flat(S1))


# ---------------- bass program ----------------
_NC_CACHE = {}


def _build_nc():
    if 'nc' in _NC_CACHE:
        return _NC_CACHE['nc']
    import concourse.bacc as bacc
    import concourse.mybir as mybir
    import concourse.tile as tile
    from contextlib import ExitStack

    f16, f32 = mybir.dt.float16, mybir.dt.float32
    RELU = mybir.ActivationFunctionType.Relu
    IDENT = mybir.ActivationFunctionType.Identity
    ADD = mybir.AluOpType.add
    XY = mybir.AxisListType.XY

    nc = bacc.Bacc("TRN2", target_bir_lowering=False, debug=False,
                   enable_asserts=False, num_devices=N_CORES)
    xin = nc.dram_tensor('xin', [24, _flat(S1)], f16, kind='ExternalInput')
    wdr = nc.dram_tensor('wts', [128, WCOLS], f16, kind='ExternalInput')
    bdr = nc.dram_tensor('bias', [128, NBIAS], f32, kind='ExternalInput')
    fdr = nc.dram_tensor('fcw', [128, 56], f32, kind='ExternalInput')
    y1d = nc.dram_tensor('y1', [20, 32], f32, kind='ExternalOutput')
    y2d = nc.dram_tensor('y2', [20, 32], f32, kind='ExternalOutput')
    gld = nc.dram_tensor('glog', [16, 8], f32, kind='ExternalOutput')

    with tile.TileContext(nc) as tc, ExitStack() as ctx:
        persist = ctx.enter_context(tc.tile_pool(name='persist', bufs=1))
        wpool = ctx.enter_context(tc.tile_pool(name='wpool', bufs=3))
        pspool = ctx.enter_context(
            tc.tile_pool(name='psum', bufs=8, space='PSUM'))
        tpool = ctx.enter_context(tc.tile_pool(name='tmp', bufs=4))

        X = persist.tile([24, _flat(S1) + 2 * GUARD], f16, tag='X')
        bufs = {}
        for nm, st in BUF_STAGE.items():
            bufs[nm] = persist.tile(
                [128, _flat(STAGES[st]) + 2 * GUARD], f16, tag=nm, name=nm)
        bt = persist.tile([128, NBIAS], f32, tag='bias')
        ft = persist.tile([128, 56], f32, tag='fcw')

        nc.sync.dma_start(bt[:], bdr[:])
        nc.sync.dma_start(ft[:], fdr[:])
        nc.sync.dma_start(X[:, GUARD:GUARD + _flat(S1)], xin[:])
        nc.gpsimd.memset(X[:, 0:GUARD], 0.0)
        nc.gpsimd.memset(X[:, GUARD + _flat(S1):], 0.0)
        for nm, st in BUF_STAGE.items():
            F = _flat(STAGES[st])
            nc.gpsimd.memset(bufs[nm][:, 0:GUARD], 0.0)
            nc.gpsimd.memset(bufs[nm][:, GUARD + F:], 0.0)

        # tap-packed staging: P0 = taps 0-3 of x, P1 = taps 4-7 (shifted
        # copies stacked in partitions); tap 8 reads X directly.
        F1 = _flat(S1)
        P0 = persist.tile([96, F1 + 2 * GUARD], f16, tag='P0', name='P0')
        P1 = persist.tile([96, F1 + 2 * GUARD], f16, tag='P1', name='P1')
        for t in range(8):
            ky, kx = t // 3, t % 3
            delta = (ky - 1) * 34 + (kx - 1)
            Pb = P0 if t < 4 else P1
            tl = t % 4
            nc.sync.dma_start(
                Pb[tl * 24:(tl + 1) * 24, GUARD:GUARD + F1],
                X[:, GUARD + delta:GUARD + delta + F1])

        def pad_memsets(dst, s):
            F = _flat(s)
            Hp, J = s['Hp'], s['J']
            v = dst[:, GUARD:GUARD + F].rearrange(
                'p (j y x) -> p j y x', j=J, y=Hp, x=Hp)
            jc = J // 4
            for j0 in range(0, J, jc):
                nc.gpsimd.memset(v[:, j0:j0 + jc, 0:Hp:Hp - 1, :], 0.0)
                nc.gpsimd.memset(v[:, j0:j0 + jc, :, 0:Hp:Hp - 1], 0.0)

        def emit_conv(u):
            s = STAGES[u['stage']]
            F, Hp = _flat(s), s['Hp']
            kin = u['kin']
            nt = u['ntaps']
            wt = wpool.tile([128, nt * 128], f16, tag='w', name='wconv')
            nc.sync.dma_start(wt[:], wdr[:, u['wcol']:u['wcol'] + nt * 128])
            IN = bufs[u['src']] if u['src'] != 'X' else X
            OUT = bufs[u['dst']]
            b_ap = bt[:, u['bcol']:u['bcol'] + 1]
            ntiles = (F + 511) // 512
            for i in range(ntiles):
                n = min(512, F - i * 512)
                ps = pspool.tile([128, 512], f32, tag='ps', name='ps')
                if u.get('packed'):
                    for p_i, (src_buf, k, off0) in enumerate(
                            ((P0, 96, 0), (P1, 96, 0), (X, 24, 35))):
                        off = GUARD + off0 + i * 512
                        nc.tensor.matmul(
                            ps[:, :n],
                            wt[0:k, p_i * 128:p_i * 128 + 128],
                            src_buf[0:k, off:off + n],
                            start=(p_i == 0), stop=(p_i == 2))
                else:
                    for t in range(9):
                        dy, dx = t // 3 - 1, t % 3 - 1
                        off = GUARD + dy * Hp + dx + i * 512
                        nc.tensor.matmul(
                            ps[:, :n],
                            wt[0:kin, t * 128:t * 128 + 128],
                            IN[0:kin, off:off + n],
                            start=(t == 0), stop=(t == 8))
                dst = OUT[:, GUARD + i * 512:GUARD + i * 512 + n]
                if u['add'] is None:
                    nc.scalar.activation(dst, ps[:, :n], RELU, bias=b_ap)
                else:
                    AD = bufs[u['add']]
                    tmp = tpool.tile([128, 512], f32, tag='tmp', name='tmp')
                    nc.vector.tensor_add(
                        tmp[:, :n], ps[:, :n],
                        AD[:, GUARD + i * 512:GUARD + i * 512 + n])
                    nc.scalar.activation(dst, tmp[:, :n], RELU, bias=b_ap)
            pad_memsets(OUT, s)

        def emit_trans(u):
            si, so = STAGES[u['si']], STAGES[u['so']]
            Ji, Ho, Hpo = si['J'], so['H'], so['Hp']
            nt = u['ntaps']
            wt = wpool.tile([128, 2 * nt * 128], f16, tag='w', name='wtrans')
            nc.sync.dma_start(wt[:], wdr[:, u['wcol']:u['wcol'] + 2 * nt * 128])
            IN, OUT = bufs[u['src']], bufs[u['dst']]
            b_ap = bt[:, u['bcol']:u['bcol'] + 1]
            vin = IN[:, GUARD:GUARD + _flat(si)].rearrange(
                'p (j y x) -> p j y x', j=Ji, y=si['Hp'], x=si['Hp'])
            vout = OUT[:, GUARD:GUARD + _flat(so)].rearrange(
                'p (j y x) -> p j y x', j=so['J'], y=Hpo, x=Hpo)
            m = 512 // (Ho * Ho)          # images per psum tile
            for st_i in range(2):          # set A then B
                for k in range(Ji // m):
                    j0 = k * m
                    n = m * Ho * Ho
                    ps = pspool.tile([128, 512], f32, tag='ps', name='ps')
                    for t in range(nt):
                        ky, kx = (t // 3, t % 3) if nt == 9 else (0, 0)
                        rhs = vin[:, j0:j0 + m,
                                  1 + ky:1 + ky + 2 * Ho - 1:2,
                                  1 + kx:1 + kx + 2 * Ho - 1:2]
                        c0 = u['wcol'] + (st_i * nt + t) * 128
                        nc.tensor.matmul(
                            ps[:, :n], wt[:, c0 - u['wcol']:c0 - u['wcol'] + 128],
                            rhs, start=(t == 0), stop=(t == nt - 1))
                    jo = st_i * Ji + j0
                    dst = vout[:, jo:jo + m, 1:1 + Ho, 1:1 + Ho]
                    nc.scalar.activation(
                        dst, ps[:, :n], RELU if u['relu'] else IDENT, bias=b_ap)
            pad_memsets(OUT, so)

        def emit_pool(e, src):
            v = bufs[src][:, GUARD:GUARD + _flat(S3)].rearrange(
                'p (j y x) -> p j y x', j=32, y=10, x=10)
            pooled = tpool.tile([128, 32], f32, tag=f'pool{e}',
                                name=f'pool{e}', bufs=1)
            nc.vector.tensor_reduce(
                pooled[:], v[:, :, 1:9, 1:9], axis=XY, op=ADD)
            return pooled

        def emit_fc(e, pooled, ydst):
            ps = pspool.tile([128, 512], f32, tag='ps', name='ps')
            nc.tensor.matmul(ps[0:20, 0:32], ft[:, e * 20:e * 20 + 20],
                             pooled[:], start=True, stop=True)
            ysb = tpool.tile([20, 32], f32, tag=f'y{e}', name=f'ysb{e}', bufs=1)
            nc.scalar.activation(
                ysb[:], ps[0:20, 0:32], IDENT,
                bias=bt[0:20, BCOL_FCB1 + e:BCOL_FCB1 + e + 1])
            nc.sync.dma_start(ydst[:], ysb[:])

        def emit_gate_pool():
            v = bufs['B'][:, GUARD:GUARD + _flat(S1)].rearrange(
                'p (j y x) -> p j y x', j=8, y=34, x=34)
            pooled = tpool.tile([128, 8], f32, tag='poolg', name='poolg',
                                bufs=1)
            nc.vector.tensor_reduce(
                pooled[:], v[:, :, 1:33, 1:33], axis=XY, op=ADD)
            return pooled

        def emit_gate_fc(pooled):
            ps = pspool.tile([128, 512], f32, tag='ps', name='ps')
            nc.tensor.matmul(ps[0:16, 0:8], ft[:, 40:56], pooled[:],
                             start=True, stop=True)
            gsb = tpool.tile([16, 8], f32, tag='yg', name='gsb', bufs=1)
            nc.scalar.activation(gsb[:], ps[0:16, 0:8], IDENT,
                                 bias=bt[0:16, BCOL_LB:BCOL_LB + 1])
            nc.sync.dma_start(gld[:], gsb[:])

        def emit_unit(u):
            if u['kind'] == 'conv':
                emit_conv(u)
            else:
                emit_trans(u)

        e1_units = [u for u in UNITS if u['tag'].startswith('e1.')]
        e2_units = [u for u in UNITS if u['tag'].startswith('e2.')]
        emit_conv(UNITS[0])            # gate conv -> B
        pooled_g = emit_gate_pool()    # DVE reduce, before B is reused
        emit_unit(e1_units[0])         # e1 conv0
        emit_gate_fc(pooled_g)         # PE reaches this well after the reduce
        for u in e1_units[1:]:
            emit_unit(u)
        pooled_1 = emit_pool(0, 'H')   # must run before e2 overwrites H
        emit_unit(e2_units[0])         # e2 conv0
        emit_fc(0, pooled_1, y1d)
        for u in e2_units[1:]:
            emit_unit(u)
        pooled_2 = emit_pool(1, 'H')
        emit_fc(1, pooled_2, y2d)

    nc.compile()
    _NC_CACHE['nc'] = nc
    return nc


# ---------------- host orchestration ----------------
def _soft_topk(logits):
    logits = logits.astype(np.float32)
    m = logits.max(-1, keepdims=True)
    e = np.exp(logits - m)
    s = e / e.sum(-1, keepdims=True)
    diff = s[:, :, None] - s[:, None, :]
    with np.errstate(over='ignore'):
        sigma = 1.0 / (1.0 + np.exp(diff / np.float32(1e-5)))
    r = 1.0 + (sigma.sum(-1) - np.float32(0.5))
    a = 1.0 / (1.0 + np.exp(-(np.float32(2.5) - r) / np.float32(1e-5)))
    return a * s


def kernel(x, train, gate_params, expert_params1, expert_params2):
    del train
    from concourse.bass_utils import run_bass_kernel_spmd

    x = np.asarray(x, np.float32)

    def tonp(tree):
        if isinstance(tree, dict):
            return {k: tonp(v) for k, v in tree.items()}
        if isinstance(tree, (list, tuple)):
            return type(tree)(tonp(v) for v in tree)
        return np.asarray(tree, np.float32)

    gp = tonp(gate_params)
    P1 = tonp(expert_params1)
    P2 = tonp(expert_params2)

    wts, bias, fcw = _build_tables(gp, P1, P2)
    nc = _build_nc()

    in_maps = []
    for ci in range(N_CORES):
        xc = x[ci * N_PER_CORE:(ci + 1) * N_PER_CORE]
        in_maps.append(dict(xin=_pack_x(xc), wts=wts, bias=bias, fcw=fcw))

    res = run_bass_kernel_spmd(nc, in_maps, core_ids=list(range(N_CORES)))
    outs = res.results

    i1, i3 = _s1_ids(), _s3_ids()
    NB = x.shape[0]
    y = np.zeros((NB, 10), np.float32)
    logits = np.zeros((NB, 2), np.float32)
    for ci in range(N_CORES):
        o = outs[ci]
        ys1 = np.zeros((N_PER_CORE, 10), np.float32)
        ys2 = np.zeros((N_PER_CORE, 10), np.float32)
        gl = np.zeros((N_PER_CORE, 2), np.float32)
        for g in range(2):
            for j in range(32):
                ys1[i3[g, j]] = o['y1'][g * 10:(g + 1) * 10, j]
                ys2[i3[g, j]] = o['y2'][g * 10:(g + 1) * 10, j]
        for g in range(8):
            for j in range(8):
                gl[i1[g, j]] = o['glog'][g * 2:(g + 1) * 2, j]
        gates = _soft_topk(gl)
        y[ci * N_PER_CORE:(ci + 1) * N_PER_CORE] = \
            gates[:, 0:1] * ys1 + gates[:, 1:2] * ys2
        logits[ci * N_PER_CORE:(ci + 1) * N_PER_CORE] = gl

    gates_all = _soft_topk(logits)
    importance = gates_all.sum(0)
    mean = importance.mean()
    var = importance.var(ddof=1)
    loss = np.float32(0.01) * var / (mean * mean + np.float32(1e-10))
    return y, np.float32(loss)


# revision 10
# speedup vs baseline: 1.6310x; 1.2237x over previous
"""Trainium2 Bass kernel for the 2-expert ResNet20 MoE (nn_MoE_48095043780864).

Strategy (pure data parallel, 8 cores x 64 images):
  - Channel counts (3/16/32/64) are far below the 128-wide PE array, so we
    pack G images' channels into the partition dim (G=8/4/2 per stage) and
    use block-diagonal stationary matrices: every conv = 9 shifted-tap
    matmuls [128,128]x[128,<=512] accumulated in PSUM fp32.
  - Activations live in SBUF as fp16, padded spatially (pad=1), flat per
    image. Matmul rhs / epilogue APs address only valid pixels (strided
    2-3 dim APs), so pad rings are zeroed once at init and never touched.
  - The three 3->16 convs (gate + 2 conv0s) use host-prestacked shifted
    input copies (taps in partitions): 2 matmul passes (K=96/120) each.
  - Stride-2 convs (jax SAME pads (0,1) on even input) are two matmul sets
    (A: j'=j from input groups g, B: j'=j+J from groups g+G') with strided
    access patterns; they also perform the G -> G/2 repacking for free.
  - BN is folded into weights (scale) + activation bias; ReLU+bias fused in
    the ScalarE epilogue; residual adds on VectorE.
  - Gate softmax/soft-topk, the gate-weighted combine and the load-balance
    loss run on host in fp32 (tiny), bit-matching the sharp sigmoids.
"""
import numpy as np

# ---------------- static plan ----------------
S1 = dict(C=16, G=8, J=8, H=32, Hp=34)
S2 = dict(C=32, G=4, J=16, H=16, Hp=18)
S3 = dict(C=64, G=2, J=32, H=8, Hp=10)
STAGES = {1: S1, 2: S2, 3: S3}
N_CORES = 8
N_PER_CORE = 64
F1 = 8 * 34 * 34


def _flat(s):
    return s['J'] * s['Hp'] * s['Hp']


def _plan():
    units = []
    wcol = [0]
    bcol = [0]

    def conv(stage, src, dst, kin, add=None, tag=''):
        packed = (kin == 24)
        u = dict(kind='conv', stage=stage, src=src, dst=dst, kin=kin, add=add,
                 ntaps=(2 if packed else 9), packed=packed,
                 wcol=wcol[0], bcol=bcol[0], tag=tag)
        wcol[0] += u['ntaps'] * 128
        bcol[0] += 1
        units.append(u)

    def trans(si, so, src, dst, ntaps, relu, tag=''):
        u = dict(kind='trans', si=si, so=so, src=src, dst=dst, ntaps=ntaps,
                 relu=relu, wcol=wcol[0], bcol=bcol[0], tag=tag)
        wcol[0] += 2 * ntaps * 128
        bcol[0] += 1
        units.append(u)

    conv(1, 'P', 'B', 24, tag='gate')
    for e in (1, 2):
        p = f'e{e}.'
        conv(1, 'P', 'A', 24, tag=p + 'conv0')
        h = 'A'
        for blk in range(3):
            conv(1, h, 'B', 128, tag=f'{p}b{blk}c1')
            nxt = 'C' if h == 'A' else 'A'
            conv(1, 'B', nxt, 128, add=h, tag=f'{p}b{blk}c2')
            h = nxt
        trans(1, 2, h, 'D', 9, True, tag=p + 'b3c1')
        trans(1, 2, h, 'F', 1, False, tag=p + 'b3proj')
        conv(2, 'D', 'E', 128, add='F', tag=p + 'b3c2')
        h = 'E'
        for blk in (4, 5):
            conv(2, h, 'D', 128, tag=f'{p}b{blk}c1')
            nxt = 'F' if h == 'E' else 'E'
            conv(2, 'D', nxt, 128, add=h, tag=f'{p}b{blk}c2')
            h = nxt
        assert h == 'E'
        trans(2, 3, h, 'G', 9, True, tag=p + 'b6c1')
        trans(2, 3, h, 'I', 1, False, tag=p + 'b6proj')
        conv(3, 'G', 'H', 128, add='I', tag=p + 'b6c2')
        h = 'H'
        for blk in (7, 8):
            conv(3, h, 'G', 128, tag=f'{p}b{blk}c1')
            nxt = 'I' if h == 'H' else 'H'
            conv(3, 'G', nxt, 128, add=h, tag=f'{p}b{blk}c2')
            h = nxt
        assert h == 'H'
    nb = bcol[0] + 3
    return units, wcol[0], nb


UNITS, WCOLS, NBIAS = _plan()
BCOL_FCB1 = NBIAS - 3
BCOL_FCB2 = NBIAS - 2
BCOL_LB = NBIAS - 1
BUF_STAGE = dict(A=1, B=1, C=1, D=2, E=2, F=2, G=3, H=3, I=3)

# per-stage conv tiling: (tiles, imgs/tile, rows/tile, cols, N)
CONV_TILES = {
    1: dict(nj=1, nh=2, rows=16, cols=32, N=512),   # 16 tiles (j, half)
    2: dict(nj=2, nh=1, rows=16, cols=16, N=512),   # 8 tiles (j-pair)
    3: dict(nj=4, nh=1, rows=8, cols=8, N=256),     # 8 tiles (j-quad)
}


def _s1_ids():
    return np.array([[8 * g + j for j in range(8)] for g in range(8)])


def _s3_ids():
    i1 = _s1_ids()
    i2 = np.zeros((4, 16), int)
    for g in range(4):
        i2[g, :8] = i1[g]
        i2[g, 8:] = i1[g + 4]
    i3 = np.zeros((2, 32), int)
    for g in range(2):
        i3[g, :16] = i2[g]
        i3[g, 16:] = i2[g + 2]
    return i3


# ---------------- host-side tables ----------------
def _np32(a):
    return np.asarray(a, np.float32)


def _fold(p, wk, gk, bk):
    w = _np32(p[wk])
    g = _np32(p[gk])
    return w * g[:, None, None, None], _np32(p[bk])


def _blockdiag(Wf, t, gmap, I_, O):
    out = np.zeros((128, 128), np.float32)
    blk = Wf[:, :, t // 3, t % 3].T
    for gi, go in gmap:
        out[gi * I_:(gi + 1) * I_, go * O:(go + 1) * O] = blk
    return out


def _build_tables(gp, P1, P2):
    wts = np.zeros((128, WCOLS), np.float32)
    bias = np.zeros((128, NBIAS), np.float32)
    fcw = np.zeros((128, 56), np.float32)

    def expert_weight_seq(P):
        W0, b0 = _fold(P, 'w0', 'g0', 'b0')
        yield ('conv', W0, b0, 8)
        blocks = P['blocks']
        for blk in range(3):
            p, _ = blocks[blk]
            yield ('conv', *_fold(p, 'w1', 'g1', 'b1'), 8)
            yield ('conv', *_fold(p, 'w2', 'g2', 'b2'), 8)
        p, _ = blocks[3]
        t12 = (4, [(g, g) for g in range(4)], [(g + 4, g) for g in range(4)])
        yield ('trans', *_fold(p, 'w1', 'g1', 'b1'), t12)
        yield ('trans', *_fold(p, 'wp', 'gp', 'bp'), t12)
        yield ('conv', *_fold(p, 'w2', 'g2', 'b2'), 4)
        for blk in (4, 5):
            p, _ = blocks[blk]
            yield ('conv', *_fold(p, 'w1', 'g1', 'b1'), 4)
            yield ('conv', *_fold(p, 'w2', 'g2', 'b2'), 4)
        p, _ = blocks[6]
        t23 = (2, [(g, g) for g in range(2)], [(g + 2, g) for g in range(2)])
        yield ('trans', *_fold(p, 'w1', 'g1', 'b1'), t23)
        yield ('trans', *_fold(p, 'wp', 'gp', 'bp'), t23)
        yield ('conv', *_fold(p, 'w2', 'g2', 'b2'), 2)
        for blk in (7, 8):
            p, _ = blocks[blk]
            yield ('conv', *_fold(p, 'w1', 'g1', 'b1'), 2)
            yield ('conv', *_fold(p, 'w2', 'g2', 'b2'), 2)

    seq = [('conv', _np32(gp['cw']), _np32(gp['cb']), 8)] \
        + list(expert_weight_seq(P1)) + list(expert_weight_seq(P2))
    assert len(seq) == len(UNITS)
    for u, (kind, Wf, bv, extra) in zip(UNITS, seq):
        assert kind == u['kind']
        O, I_ = Wf.shape[0], Wf.shape[1]
        if kind == 'conv':
            G = extra
            gmap = [(g, g) for g in range(G)]
            if u.get('packed'):
                # pass0: taps 0-3 at rows t*24+(g,c); pass1: taps 4-7 + tap8
                # at rows 96..119
                for p_i in range(2):
                    m = np.zeros((128, 128), np.float32)
                    for tl in range(4):
                        t = p_i * 4 + tl
                        blk = Wf[:, :, t // 3, t % 3].T
                        for g in range(8):
                            m[tl * 24 + g * 3:tl * 24 + (g + 1) * 3,
                              g * 16:(g + 1) * 16] = blk
                    if p_i == 1:
                        blk = Wf[:, :, 2, 2].T
                        for g in range(8):
                            m[96 + g * 3:96 + (g + 1) * 3,
                              g * 16:(g + 1) * 16] = blk
                    wts[:, u['wcol'] + p_i * 128:
                        u['wcol'] + (p_i + 1) * 128] = m
            else:
                for t in range(9):
                    wts[:, u['wcol'] + t * 128:u['wcol'] + (t + 1) * 128] = \
                        _blockdiag(Wf, t, gmap, I_, O)
            bias[:, u['bcol']] = np.tile(bv, G)
        else:
            Gout, gA, gB = extra
            nt = u['ntaps']
            for t in range(nt):
                tt = t if nt == 9 else 0
                wts[:, u['wcol'] + t * 128:u['wcol'] + (t + 1) * 128] = \
                    _blockdiag(Wf, tt, gA, I_, O)
                c0 = u['wcol'] + (nt + t) * 128
                wts[:, c0:c0 + 128] = _blockdiag(Wf, tt, gB, I_, O)
            bias[:, u['bcol']] = np.tile(bv, Gout)

    for e, P in ((0, P1), (1, P2)):
        fw = _np32(P['fcw']) / 64.0
        for g in range(2):
            fcw[g * 64:(g + 1) * 64, e * 20 + g * 10:e * 20 + (g + 1) * 10] = fw.T
        bias[:20, BCOL_FCB1 + e] = np.tile(_np32(P['fcb']), 2)
    lw = _np32(gp['lw']) / 1024.0
    for g in range(8):
        fcw[g * 16:(g + 1) * 16, 40 + g * 2:40 + (g + 1) * 2] = lw.T
    bias[:16, BCOL_LB] = np.tile(_np32(gp['lb']), 8)
    return wts.astype(np.float16), bias, fcw


def _pack_x(x_core):
    """[64,3,32,32] -> (p0 [96,F1], p1 [120,F1]) fp16: host-shifted tap
    stacks. p0 rows = taps 0-3 x (g,c); p1 rows 0-95 = taps 4-7, rows
    96-119 = tap 8."""
    xf = np.zeros((24, 8, 34, 34), np.float16)
    ids = _s1_ids()
    for g in range(8):
        xf[g * 3:(g + 1) * 3, :, 1:33, 1:33] = \
            x_core[ids[g]].transpose(1, 0, 2, 3)
    xf = xf.reshape(24, F1)

    def shift(d):
        out = np.zeros((24, F1), np.float16)
        if d >= 0:
            out[:, :F1 - d] = xf[:, d:]
        else:
            out[:, -d:] = xf[:, :F1 + d]
        return out

    p0 = np.zeros((96, F1), np.float16)
    p1 = np.zeros((120, F1), np.float16)
    for t in range(9):
        ky, kx = t // 3, t % 3
        d = (ky - 1) * 34 + (kx - 1)
        s = shift(d)
        if t < 4:
            p0[t * 24:(t + 1) * 24] = s
        elif t < 8:
            p1[(t - 4) * 24:(t - 3) * 24] = s
        else:
            p1[96:120] = s
    return p0, p1


# ---------------- bass program ----------------
_NC_CACHE = {}


def _build_nc():
    if 'nc' in _NC_CACHE:
        return _NC_CACHE['nc']
    import concourse.bacc as bacc
    import concourse.mybir as mybir
    import concourse.tile as tile
    from contextlib import ExitStack

    f16, f32 = mybir.dt.float16, mybir.dt.float32
    RELU = mybir.ActivationFunctionType.Relu
    IDENT = mybir.ActivationFunctionType.Identity
    ADD = mybir.AluOpType.add
    XY = mybir.AxisListType.XY

    nc = bacc.Bacc("TRN2", target_bir_lowering=False, debug=False,
                   enable_asserts=False, num_devices=N_CORES)
    p0d = nc.dram_tensor('p0', [96, F1], f16, kind='ExternalInput')
    p1d = nc.dram_tensor('p1', [120, F1], f16, kind='ExternalInput')
    wdr = nc.dram_tensor('wts', [128, WCOLS], f16, kind='ExternalInput')
    bdr = nc.dram_tensor('bias', [128, NBIAS], f32, kind='ExternalInput')
    fdr = nc.dram_tensor('fcw', [128, 56], f32, kind='ExternalInput')
    y1d = nc.dram_tensor('y1', [20, 32], f32, kind='ExternalOutput')
    y2d = nc.dram_tensor('y2', [20, 32], f32, kind='ExternalOutput')
    gld = nc.dram_tensor('glog', [16, 8], f32, kind='ExternalOutput')

    with tile.TileContext(nc) as tc, ExitStack() as ctx:
        persist = ctx.enter_context(tc.tile_pool(name='persist', bufs=1))
        wpool = ctx.enter_context(tc.tile_pool(name='wpool', bufs=3))
        pspool = ctx.enter_context(
            tc.tile_pool(name='psum', bufs=8, space='PSUM'))
        tpool = ctx.enter_context(tc.tile_pool(name='tmp', bufs=4))

        P0 = persist.tile([96, F1], f16, tag='P0', name='P0')
        P1 = persist.tile([120, F1], f16, tag='P1', name='P1')
        bufs = {}
        for nm, st in BUF_STAGE.items():
            bufs[nm] = persist.tile([128, _flat(STAGES[st])], f16,
                                    tag=nm, name=nm)
        bt = persist.tile([128, NBIAS], f32, tag='bias', name='bt')
        ft = persist.tile([128, 56], f32, tag='fcw', name='ft')

        nc.sync.dma_start(bt[:], bdr[:])
        nc.sync.dma_start(ft[:], fdr[:])
        nc.sync.dma_start(P0[:], p0d[:])
        nc.sync.dma_start(P1[:], p1d[:])

        # zero the pad rings once; valid pixels are always overwritten and
        # pads are never written again.
        for nm, st in BUF_STAGE.items():
            s = STAGES[st]
            Hp, J = s['Hp'], s['J']
            v = bufs[nm].rearrange('p (j y x) -> p j y x', j=J, y=Hp, x=Hp)
            nc.gpsimd.memset(v[:, :, 0:Hp:Hp - 1, :], 0.0)
            nc.gpsimd.memset(v[:, :, :, 0:Hp:Hp - 1], 0.0)

        def conv_aps(stage, buf, ti, dy=0, dx=0):
            """valid-pixel AP of tile ti of `buf` shifted by (dy, dx)."""
            s = STAGES[stage]
            ct = CONV_TILES[stage]
            Hp = s['Hp']
            v = buf.rearrange('p (j y x) -> p j y x', j=s['J'], y=Hp, x=Hp)
            nj, nh, rows = ct['nj'], ct['nh'], ct['rows']
            j0 = (ti // nh) * nj
            r0 = (ti % nh) * rows
            return v[:, j0:j0 + nj,
                     1 + r0 + dy:1 + r0 + dy + rows,
                     1 + dx:1 + dx + ct['cols']]

        def emit_conv(u):
            st = u['stage']
            s = STAGES[st]
            ct = CONV_TILES[st]
            nt = u['ntaps']
            N = ct['N']
            ntiles = (s['J'] // ct['nj']) * ct['nh']
            wt = wpool.tile([128, nt * 128], f16, tag='w', name='wconv')
            nc.sync.dma_start(wt[:], wdr[:, u['wcol']:u['wcol'] + nt * 128])
            OUT = bufs[u['dst']]
            b_ap = bt[:, u['bcol']:u['bcol'] + 1]
            for ti in range(ntiles):
                ps = pspool.tile([128, 512], f32, tag='ps', name='ps')
                if u.get('packed'):
                    for p_i, (src_buf, k) in enumerate(((P0, 96), (P1, 120))):
                        nc.tensor.matmul(
                            ps[:, :N],
                            wt[0:k, p_i * 128:p_i * 128 + 128],
                            conv_aps(st, src_buf[0:k], ti),
                            start=(p_i == 0), stop=(p_i == 1))
                else:
                    IN = bufs[u['src']]
                    for t in range(9):
                        dy, dx = t // 3 - 1, t % 3 - 1
                        nc.tensor.matmul(
                            ps[:, :N],
                            wt[:, t * 128:t * 128 + 128],
                            conv_aps(st, IN, ti, dy, dx),
                            start=(t == 0), stop=(t == 8))
                dst = conv_aps(st, OUT, ti)
                if u['add'] is None:
                    nc.scalar.activation(dst, ps[:, :N], RELU, bias=b_ap)
                else:
                    AD = bufs[u['add']]
                    tmp = tpool.tile([128, 512], f32, tag='tmp', name='tmp')
                    nc.vector.tensor_add(tmp[:, :N], ps[:, :N],
                                         conv_aps(st, AD, ti))
                    nc.scalar.activation(dst, tmp[:, :N], RELU, bias=b_ap)

        def emit_trans(u):
            si, so = STAGES[u['si']], STAGES[u['so']]
            Ji, Ho, Hpo = si['J'], so['H'], so['Hp']
            nt = u['ntaps']
            wt = wpool.tile([128, 2 * nt * 128], f16, tag='w', name='wtrans')
            nc.sync.dma_start(wt[:], wdr[:, u['wcol']:u['wcol'] + 2 * nt * 128])
            IN, OUT = bufs[u['src']], bufs[u['dst']]
            b_ap = bt[:, u['bcol']:u['bcol'] + 1]
            vin = IN.rearrange('p (j y x) -> p j y x',
                               j=Ji, y=si['Hp'], x=si['Hp'])
            vout = OUT.rearrange('p (j y x) -> p j y x',
                                 j=so['J'], y=Hpo, x=Hpo)
            m = 512 // (Ho * Ho)
            for st_i in range(2):
                for k in range(Ji // m):
                    j0 = k * m
                    n = m * Ho * Ho
                    ps = pspool.tile([128, 512], f32, tag='ps', name='ps')
                    for t in range(nt):
                        ky, kx = (t // 3, t % 3) if nt == 9 else (0, 0)
                        rhs = vin[:, j0:j0 + m,
                                  1 + ky:1 + ky + 2 * Ho - 1:2,
                                  1 + kx:1 + kx + 2 * Ho - 1:2]
                        c0 = (st_i * nt + t) * 128
                        nc.tensor.matmul(
                            ps[:, :n], wt[:, c0:c0 + 128], rhs,
                            start=(t == 0), stop=(t == nt - 1))
                    jo = st_i * Ji + j0
                    dst = vout[:, jo:jo + m, 1:1 + Ho, 1:1 + Ho]
                    nc.scalar.activation(
                        dst, ps[:, :n], RELU if u['relu'] else IDENT,
                        bias=b_ap)

        def emit_pool(e, src):
            v = bufs[src].rearrange('p (j y x) -> p j y x', j=32, y=10, x=10)
            pooled = tpool.tile([128, 32], f32, tag=f'pool{e}',
                                name=f'pool{e}', bufs=1)
            for j0 in range(0, 32, 8):
                nc.vector.tensor_reduce(
                    pooled[:, j0:j0 + 8], v[:, j0:j0 + 8, 1:9, 1:9],
                    axis=XY, op=ADD)
            return pooled

        def emit_fc(e, pooled, ydst):
            ps = pspool.tile([128, 512], f32, tag='ps', name='ps')
            nc.tensor.matmul(ps[0:20, 0:32], ft[:, e * 20:e * 20 + 20],
                             pooled[:], start=True, stop=True)
            ysb = tpool.tile([20, 32], f32, tag=f'y{e}', name=f'ysb{e}',
                             bufs=1)
            nc.scalar.activation(
                ysb[:], ps[0:20, 0:32], IDENT,
                bias=bt[0:20, BCOL_FCB1 + e:BCOL_FCB1 + e + 1])
            nc.sync.dma_start(ydst[:], ysb[:])

        def emit_gate_pool():
            v = bufs['B'].rearrange('p (j y x) -> p j y x', j=8, y=34, x=34)
            pooled = tpool.tile([128, 8], f32, tag='poolg', name='poolg',
                                bufs=1)
            for j0 in range(0, 8, 2):
                nc.vector.tensor_reduce(
                    pooled[:, j0:j0 + 2], v[:, j0:j0 + 2, 1:33, 1:33],
                    axis=XY, op=ADD)
            return pooled

        def emit_gate_fc(pooled):
            ps = pspool.tile([128, 512], f32, tag='ps', name='ps')
            nc.tensor.matmul(ps[0:16, 0:8], ft[:, 40:56], pooled[:],
                             start=True, stop=True)
            gsb = tpool.tile([16, 8], f32, tag='yg', name='gsb', bufs=1)
            nc.scalar.activation(gsb[:], ps[0:16, 0:8], IDENT,
                                 bias=bt[0:16, BCOL_LB:BCOL_LB + 1])
            nc.sync.dma_start(gld[:], gsb[:])

        def emit_unit(u):
            if u['kind'] == 'conv':
                emit_conv(u)
            else:
                emit_trans(u)

        e1_units = [u for u in UNITS if u['tag'].startswith('e1.')]
        e2_units = [u for u in UNITS if u['tag'].startswith('e2.')]
        emit_conv(UNITS[0])            # gate conv -> B
        pooled_g = emit_gate_pool()    # DVE reduce, before B is reused
        emit_unit(e1_units[0])         # e1 conv0
        emit_gate_fc(pooled_g)         # PE reaches this well after the reduce
        for u in e1_units[1:]:
            emit_unit(u)
        pooled_1 = emit_pool(0, 'H')   # must run before e2 overwrites H
        emit_unit(e2_units[0])         # e2 conv0
        emit_fc(0, pooled_1, y1d)
        for u in e2_units[1:]:
            emit_unit(u)
        pooled_2 = emit_pool(1, 'H')
        emit_fc(1, pooled_2, y2d)

    nc.compile()
    _NC_CACHE['nc'] = nc
    return nc


# ---------------- host orchestration ----------------
def _soft_topk(logits):
    logits = logits.astype(np.float32)
    m = logits.max(-1, keepdims=True)
    e = np.exp(logits - m)
    s = e / e.sum(-1, keepdims=True)
    diff = s[:, :, None] - s[:, None, :]
    with np.errstate(over='ignore'):
        sigma = 1.0 / (1.0 + np.exp(diff / np.float32(1e-5)))
    r = 1.0 + (sigma.sum(-1) - np.float32(0.5))
    a = 1.0 / (1.0 + np.exp(-(np.float32(2.5) - r) / np.float32(1e-5)))
    return a * s


def kernel(x, train, gate_params, expert_params1, expert_params2):
    del train
    from concourse.bass_utils import run_bass_kernel_spmd

    x = np.asarray(x, np.float32)

    def tonp(tree):
        if isinstance(tree, dict):
            return {k: tonp(v) for k, v in tree.items()}
        if isinstance(tree, (list, tuple)):
            return type(tree)(tonp(v) for v in tree)
        return np.asarray(tree, np.float32)

    gp = tonp(gate_params)
    P1 = tonp(expert_params1)
    P2 = tonp(expert_params2)

    wts, bias, fcw = _build_tables(gp, P1, P2)
    nc = _build_nc()

    in_maps = []
    for ci in range(N_CORES):
        p0, p1 = _pack_x(x[ci * N_PER_CORE:(ci + 1) * N_PER_CORE])
        in_maps.append(dict(p0=p0, p1=p1, wts=wts, bias=bias, fcw=fcw))

    res = run_bass_kernel_spmd(nc, in_maps, core_ids=list(range(N_CORES)))
    outs = res.results

    i1, i3 = _s1_ids(), _s3_ids()
    NB = x.shape[0]
    y = np.zeros((NB, 10), np.float32)
    logits = np.zeros((NB, 2), np.float32)
    for ci in range(N_CORES):
        o = outs[ci]
        ys1 = np.zeros((N_PER_CORE, 10), np.float32)
        ys2 = np.zeros((N_PER_CORE, 10), np.float32)
        gl = np.zeros((N_PER_CORE, 2), np.float32)
        for g in range(2):
            for j in range(32):
                ys1[i3[g, j]] = o['y1'][g * 10:(g + 1) * 10, j]
                ys2[i3[g, j]] = o['y2'][g * 10:(g + 1) * 10, j]
        for g in range(8):
            for j in range(8):
                gl[i1[g, j]] = o['glog'][g * 2:(g + 1) * 2, j]
        gates = _soft_topk(gl)
        y[ci * N_PER_CORE:(ci + 1) * N_PER_CORE] = \
            gates[:, 0:1] * ys1 + gates[:, 1:2] * ys2
        logits[ci * N_PER_CORE:(ci + 1) * N_PER_CORE] = gl

    gates_all = _soft_topk(logits)
    importance = gates_all.sum(0)
    mean = importance.mean()
    var = importance.var(ddof=1)
    loss = np.float32(0.01) * var / (mean * mean + np.float32(1e-10))
    return y, np.float32(loss)
